# revision 13
# baseline (speedup 1.0000x reference)
"""DiT block kernel for Trainium2, data-parallel over batch across 8 NeuronCores.

Sharding: one batch element per core (B=8, n_cores=8), zero collectives.

Layout strategy:
  - residual x kept natural (tokens on partitions, C free) in fp32
  - LN stats via bn_stats; normalized x is PE-transposed; modulate's
    per-channel shift/scale become per-partition scalars fused into the
    transpose eviction (ACT Identity scale/bias APs)
  - q^T,k^T via out^T = W.T @ h^T (lhsT=W natural); v natural via lhsT=h^T
  - scores built transposed S^T (keys on partitions, queries free): exp is
    the PSUM eviction (scale=D^-0.5 fused), softmax denominator comes from a
    ones-augmented 65th column of V in the AV matmul, CA key mask is the
    per-partition exp bias
  - K=64 score matmuls for head pairs sit at base partitions 0/64 and are
    emitted back-to-back -> PE row-group concurrency
  - weights host-packed into contiguous chunks for single-descriptor DMAs
  - MLP: fc1 transposed out (gelu-tanh eviction), fc2 natural (+residual)
"""

import numpy as np
import ml_dtypes
from contextlib import ExitStack

import concourse.mybir as mybir
import concourse.tile as tile
from concourse import bacc
from concourse.masks import make_identity

F32 = mybir.dt.float32
BF16 = mybir.dt.bfloat16
AF = mybir.ActivationFunctionType
ALU = mybir.AluOpType

B, N, C, H, D, M, MLPD, P = 8, 1024, 1024, 16, 64, 128, 4096, 128
NT = N // P       # 8 token tiles
CT = C // P       # 8 channel tiles
MT = MLPD // P    # 32 mlp tiles
EPS = 1e-6
SCALE = D ** -0.5
N_CORES = 8
NCHK = N // 512   # 2 query chunks


def build_kernel(repeat=1, gelu_composite=False):
    nc = bacc.Bacc("TRN2", target_bir_lowering=False, debug=False)

    x_d = nc.dram_tensor("x", [N, C], F32, kind="ExternalInput").ap()
    cdino_d = nc.dram_tensor("c_dino", [C], F32, kind="ExternalInput").ap()
    scr_ada = nc.dram_tensor("scr_ada", [6 * C], F32, kind="Internal").ap()
    ctextT_d = nc.dram_tensor("c_textT", [C, M], BF16,
                              kind="ExternalInput").ap()
    maskb_d = nc.dram_tensor("mask_bias", [M, 1], F32,
                             kind="ExternalInput").ap()
    # host-packed weights (see _prep_in_maps):
    #  Wqk (4, C, 512) chunk-major: [q0-3, k0-3, q4-7, k4-7]
    #  Wfc1 (8, C, 512) chunk-major
    wada_d = nc.dram_tensor("Wada", [C, 6 * C], BF16, kind="ExternalInput").ap()
    wqk_d = nc.dram_tensor("Wqk", [4, C, 512], BF16, kind="ExternalInput").ap()
    wv_d = nc.dram_tensor("Wv", [C, C], BF16, kind="ExternalInput").ap()
    wpsa_d = nc.dram_tensor("Wpsa", [C, C], BF16, kind="ExternalInput").ap()
    wq_d = nc.dram_tensor("Wq", [C, C], BF16, kind="ExternalInput").ap()
    wkv_d = nc.dram_tensor("Wkv", [C, 2 * C], BF16, kind="ExternalInput").ap()
    wpca_d = nc.dram_tensor("Wpca", [C, C], BF16, kind="ExternalInput").ap()
    wfc1_d = nc.dram_tensor("Wfc1", [8, C, 512], BF16,
                            kind="ExternalInput").ap()
    wfc2_d = nc.dram_tensor("Wfc2", [MLPD, C], BF16, kind="ExternalInput").ap()
    out_d = nc.dram_tensor("out", [N, C], F32, kind="ExternalOutput").ap()

    with tile.TileContext(nc) as tc:
      with ExitStack() as ctx:
        const_pool = ctx.enter_context(tc.tile_pool(name="const", bufs=1))
        x_pool = ctx.enter_context(tc.tile_pool(name="x", bufs=1))
        stats_pool = ctx.enter_context(tc.tile_pool(name="stats", bufs=4))
        pmm = ctx.enter_context(tc.tile_pool(name="pmm", bufs=2, space="PSUM"))
        ps_s = ctx.enter_context(tc.tile_pool(name="ps_s", bufs=4,
                                              space="PSUM"))
        ps_av = ctx.enter_context(tc.tile_pool(name="ps_av", bufs=2,
                                               space="PSUM"))

        identity = const_pool.tile([P, P], BF16, name="identity")
        make_identity(nc, identity[:])
        maskb = const_pool.tile([M, 1], F32, name="maskb")
        ctT = const_pool.tile([P, CT, M], BF16, name="ctT")
        cd_cols = const_pool.tile([P, CT], F32, name="cd_cols")
        sig_cols = const_pool.tile([P, CT], F32, name="sig_cols")
        sil_cols = const_pool.tile([P, CT], BF16, name="sil_cols")
        ada_row = const_pool.tile([1, 6 * C], F32, name="ada_row")
        ada_cols = const_pool.tile([P, 48], F32, name="ada_cols")
        eps_col = const_pool.tile([P, 1], F32, name="eps_col")
        nc.any.memset(eps_col[:], EPS)
        kcT = const_pool.tile([P, CT, M], BF16, name="kcT")
        vca = const_pool.tile([P, H, 65], BF16, name="vca")

        x_sb = [x_pool.tile([P, C], F32, name=f"x_{tt}") for tt in range(NT)]

        def ln_mod_transpose(stage, xn_pool, hT):
            """LN(x) -> transpose -> modulate; fills hT (128, CT, N) bf16."""
            xn_tiles = []
            for tt in range(NT):
                stats = stats_pool.tile([P, 2, 6], F32, tag="st",
                                        name=f"st{stage}_{tt}")
                nc.vector.bn_stats(out=stats[:, 0, :], in_=x_sb[tt][:, 0:512])
                nc.vector.bn_stats(out=stats[:, 1, :],
                                   in_=x_sb[tt][:, 512:1024])
                mv = stats_pool.tile([P, 2], F32, tag="mv",
                                     name=f"mv{stage}_{tt}")
                nc.vector.bn_aggr(out=mv[:], in_=stats[:])
                std = stats_pool.tile([P, 1], F32, tag="sd",
                                      name=f"sd{stage}_{tt}")
                nc.scalar.activation(std[:], mv[:, 1:2], AF.Sqrt,
                                     bias=eps_col[:])
                rstd = stats_pool.tile([P, 1], F32, tag="rs",
                                       name=f"rs{stage}_{tt}")
                nc.vector.reciprocal(rstd[:], std[:])
                xn = xn_pool.tile([P, C], BF16, name=f"xn{stage}_{tt}")
                nc.vector.tensor_scalar(
                    out=xn[:], in0=x_sb[tt][:], scalar1=mv[:, 0:1],
                    scalar2=rstd[:], op0=ALU.subtract, op1=ALU.mult)
                xn_tiles.append(xn)
            for ct in range(CT):
                sh_col = ada_cols[:, stage * 16 + ct: stage * 16 + ct + 1]
                sc_col = ada_cols[:, stage * 16 + 8 + ct:
                                  stage * 16 + 8 + ct + 1]
                for tg in range(NT // 4):
                    pt = pmm.tile([P, 512], BF16, tag="pmm",
                                  name=f"ptr{stage}_{ct}_{tg}")
                    for j in range(4):
                        tt = tg * 4 + j
                        nc.tensor.transpose(
                            pt[:, j * 128:(j + 1) * 128],
                            xn_tiles[tt][:, ct * 128:(ct + 1) * 128],
                            identity[:])
                    nc.scalar.activation(
                        hT[:, ct, tg * 512:(tg + 1) * 512], pt[:],
                        AF.Identity, bias=sh_col, scale=sc_col)

        def emit():
            for tt in range(NT):
                nc.sync.dma_start(out=x_sb[tt][:],
                                  in_=x_d[tt * 128:(tt + 1) * 128, :])
            nc.sync.dma_start(out=maskb[:], in_=maskb_d)
            for ct in range(CT):
                nc.sync.dma_start(out=ctT[:, ct, :],
                                  in_=ctextT_d[ct * 128:(ct + 1) * 128, :])

            # ---------------- ada = silu(c_dino) @ W_ada ----------------
            # K=32 row-packed matmuls (4 concurrent row-groups)
            nc.sync.dma_start(out=cd_cols[:],
                              in_=cdino_d.rearrange("(t p) -> p t", p=P))
            nc.scalar.activation(sig_cols[:], cd_cols[:], AF.Sigmoid)
            nc.vector.tensor_mul(sil_cols[:], cd_cols[:], sig_cols[:])
            with tc.tile_pool(name="wada", bufs=2) as wada_pool:
                for cg in range(3):
                    wt = wada_pool.tile([P, CT, 2048], BF16, tag="wada",
                                        name=f"wada{cg}")
                    for kt in range(CT):
                        nc.sync.dma_start(
                            out=wt[:, kt, :],
                            in_=wada_d[kt * 128:(kt + 1) * 128,
                                       cg * 2048:(cg + 1) * 2048])
                    for j in range(4):
                        nchnk = cg * 4 + j
                        ps = ps_s.tile([1, 512], F32, tag="ps_s",
                                       name=f"psada{nchnk}")
                        for kt in range(CT):
                            nc.tensor.matmul(
                                ps[:], sil_cols[:, kt:kt + 1],
                                wt[:, kt, j * 512:(j + 1) * 512],
                                start=(kt == 0), stop=(kt == CT - 1))
                        nc.vector.tensor_copy(
                            ada_row[:, nchnk * 512:(nchnk + 1) * 512], ps[:])
            nc.sync.dma_start(out=scr_ada.rearrange("(o c) -> o c", o=1),
                              in_=ada_row[:])
            nc.sync.dma_start(out=ada_cols[:],
                              in_=scr_ada.rearrange("(t p) -> p t", p=P))
            for s in range(3):
                blk = ada_cols[:, s * 16 + 8: s * 16 + 16]
                nc.vector.tensor_scalar_add(out=blk, in0=blk, scalar1=1.0)

            # ====== CA k/v precompute (needs only inputs; fills PE idle) ====
            nc.any.memset(vca[:, :, 64:65], 1.0)
            with tc.tile_pool(name="wkv", bufs=1) as wkv_pool:
                wkv = wkv_pool.tile([P, CT, 2 * C], BF16, name="wkv")
                for kt in range(CT):
                    nc.sync.dma_start(
                        out=wkv[:, kt, :],
                        in_=wkv_d[kt * 128:(kt + 1) * 128, :])
                for m in range(CT):
                    ps = pmm.tile([P, M], F32, tag="pmm", name=f"pskc{m}")
                    for kt in range(CT):
                        nc.tensor.matmul(
                            ps[:], wkv[:, kt, m * 128:(m + 1) * 128],
                            ctT[:, kt, :],
                            start=(kt == 0), stop=(kt == CT - 1))
                    nc.any.tensor_copy(kcT[:, m, :], ps[:])
                for nchk in range(NCHK):
                    ps = pmm.tile([P, 512], F32, tag="pmm", name=f"psvc{nchk}")
                    for kt in range(CT):
                        nc.tensor.matmul(
                            ps[:], ctT[:, kt, :],
                            wkv[:, kt, C + nchk * 512:C + (nchk + 1) * 512],
                            start=(kt == 0), stop=(kt == CT - 1))
                    nc.any.tensor_copy(
                        vca[:, 8 * nchk:8 * (nchk + 1), 0:64], ps[:])

            # ================= self-attention =================
            with tc.tile_pool(name="qT", bufs=1) as qT_pool, \
                 tc.tile_pool(name="kT", bufs=1) as kT_pool, \
                 tc.tile_pool(name="vau", bufs=1) as vau_pool, \
                 tc.tile_pool(name="attnT", bufs=1) as attnT_pool:
                qT_t = [qT_pool.tile([P, N], BF16, name=f"qT{ct}")
                        for ct in range(CT)]
                kT_t = [kT_pool.tile([P, N], BF16, name=f"kT{ct}")
                        for ct in range(CT)]
                vau_t = [vau_pool.tile([P, H, 65], BF16, name=f"vau{kt}")
                         for kt in range(NT)]
                saT_t = [attnT_pool.tile([P, N], BF16, name=f"saT{ct}")
                         for ct in range(CT)]
                for kt in range(NT):
                    nc.any.memset(vau_t[kt][:, :, 64:65], 1.0)

                with tc.tile_pool(name="xn1", bufs=1) as xn_pool, \
                     tc.tile_pool(name="hT1", bufs=1) as hT_pool:
                    hT = hT_pool.tile([P, CT, N], BF16, name="hT1")
                    ln_mod_transpose(0, xn_pool, hT)

                    with tc.tile_pool(name="wqk", bufs=2) as wqk_pool, \
                         tc.tile_pool(name="wv", bufs=1) as wv_pool:
                        # v first (natural layout)
                        wv = wv_pool.tile([P, CT, C], BF16, name="wv")
                        for kt in range(CT):
                            nc.sync.dma_start(
                                out=wv[:, kt, :],
                                in_=wv_d[kt * 128:(kt + 1) * 128, :])
                        for tt in range(NT):
                            for nchk in range(NCHK):
                                ps = pmm.tile([P, 512], F32, tag="pmm",
                                              name=f"psv{tt}_{nchk}")
                                for kt in range(CT):
                                    nc.tensor.matmul(
                                        ps[:],
                                        hT[:, kt, tt * 128:(tt + 1) * 128],
                                        wv[:, kt,
                                           nchk * 512:(nchk + 1) * 512],
                                        start=(kt == 0), stop=(kt == CT - 1))
                                nc.any.tensor_copy(
                                    vau_t[tt][:, 8 * nchk:8 * (nchk + 1),
                                              0:64], ps[:])
                        # q,k chunk-major: chunk order [q0-3, k0-3, q4-7,
                        # k4-7] so head pairs become ready early
                        for cg in range(4):
                            wt = wqk_pool.tile([P, CT, 512], BF16, tag="wqk",
                                               name=f"wqk{cg}")
                            for kt in range(CT):
                                nc.sync.dma_start(
                                    out=wt[:, kt, :],
                                    in_=wqk_d[cg, kt * 128:(kt + 1) * 128, :])
                            is_k = cg % 2 == 1
                            ct0 = (cg // 2) * 4
                            for mloc in range(4):
                                ct = ct0 + mloc
                                dst = kT_t[ct] if is_k else qT_t[ct]
                                for nchk in range(NCHK):
                                    ps = pmm.tile(
                                        [P, 512], F32, tag="pmm",
                                        name=f"psqk{cg}_{mloc}_{nchk}")
                                    for kt in range(CT):
                                        nc.tensor.matmul(
                                            ps[:],
                                            wt[:, kt,
                                               mloc * 128:(mloc + 1) * 128],
                                            hT[:, kt,
                                               nchk * 512:(nchk + 1) * 512],
                                            start=(kt == 0),
                                            stop=(kt == CT - 1))
                                    nc.any.tensor_copy(
                                        dst[:, nchk * 512:(nchk + 1) * 512],
                                        ps[:])

                # attention: head pairs (2ct, 2ct+1) share channel tile ct
                with tc.tile_pool(name="PT", bufs=20) as pt_pool, \
                     tc.tile_pool(name="rcp", bufs=4) as rcp_pool, \
                     tc.tile_pool(name="bcs", bufs=4) as bcs_pool:
                    for ct in range(CT):
                        for qc in range(NCHK):
                            pts = {0: [], 1: []}
                            for kt in range(NT):
                                for hp in range(2):
                                    pb = hp * 64
                                    ps = ps_s.tile(
                                        [P, 512], F32, tag="ps_s",
                                        name=f"S{ct}_{qc}_{kt}_{hp}")
                                    nc.tensor.matmul(
                                        ps[:],
                                        kT_t[ct][pb:pb + 64,
                                                 kt * 128:(kt + 1) * 128],
                                        qT_t[ct][pb:pb + 64,
                                                 qc * 512:(qc + 1) * 512],
                                        start=True, stop=True)
                                    pt = pt_pool.tile(
                                        [P, 512], BF16, tag="pt",
                                        name=f"P{ct}_{qc}_{kt}_{hp}")
                                    nc.scalar.activation(pt[:], ps[:],
                                                         AF.Exp, scale=SCALE)
                                    pts[hp].append(pt)
                            for hp in range(2):
                                h = 2 * ct + hp
                                pb = hp * 64
                                av = ps_av.tile([65, 512], F32, tag="ps_av",
                                                name=f"av{h}_{qc}")
                                for kt in range(NT):
                                    nc.tensor.matmul(
                                        av[:], vau_t[kt][:, h, :],
                                        pts[hp][kt][:],
                                        start=(kt == 0), stop=(kt == NT - 1))
                                rc = rcp_pool.tile([1, 512], F32, tag="rc",
                                                   name=f"rc{h}_{qc}")
                                nc.vector.reciprocal(rc[:], av[64:65, :])
                                bc = bcs_pool.tile([64, 512], F32, tag="bc",
                                                   name=f"bc{h}_{qc}")
                                nc.gpsimd.partition_broadcast(bc[:], rc[:])
                                nc.vector.tensor_mul(
                                    saT_t[ct][pb:pb + 64,
                                              qc * 512:(qc + 1) * 512],
                                    av[0:64, :], bc[:])

                with tc.tile_pool(name="wpsa", bufs=1) as wpsa_pool:
                    wp = wpsa_pool.tile([P, CT, C], BF16, name="wpsa")
                    for kt in range(CT):
                        nc.sync.dma_start(
                            out=wp[:, kt, :],
                            in_=wpsa_d[kt * 128:(kt + 1) * 128, :])
                    for tt in range(NT):
                        for nchk in range(NCHK):
                            ps = pmm.tile([P, 512], F32, tag="pmm",
                                          name=f"psp{tt}_{nchk}")
                            for kt in range(CT):
                                nc.tensor.matmul(
                                    ps[:],
                                    saT_t[kt][:, tt * 128:(tt + 1) * 128],
                                    wp[:, kt, nchk * 512:(nchk + 1) * 512],
                                    start=(kt == 0), stop=(kt == CT - 1))
                            sl = slice(nchk * 512, (nchk + 1) * 512)
                            nc.vector.tensor_add(x_sb[tt][:, sl],
                                                 x_sb[tt][:, sl], ps[:])

            # ================= cross-attention =================
            with tc.tile_pool(name="qcT", bufs=1) as qcT_pool, \
                 tc.tile_pool(name="attnT2", bufs=1) as attnT2_pool:
                qcT_t = [qcT_pool.tile([P, N], BF16, name=f"qcT{ct}")
                         for ct in range(CT)]
                caT_t = [attnT2_pool.tile([P, N], BF16, name=f"caT{ct}")
                         for ct in range(CT)]

                with tc.tile_pool(name="xn2", bufs=1) as xn_pool, \
                     tc.tile_pool(name="hT2", bufs=1) as hT_pool:
                    hT = hT_pool.tile([P, CT, N], BF16, name="hT2")
                    ln_mod_transpose(1, xn_pool, hT)
                    with tc.tile_pool(name="wq2", bufs=2) as wq_pool:
                        for mg in range(2):
                            wt = wq_pool.tile([P, CT, 512], BF16, tag="wq2",
                                              name=f"wq2_{mg}")
                            for kt in range(CT):
                                nc.sync.dma_start(
                                    out=wt[:, kt, :],
                                    in_=wq_d[kt * 128:(kt + 1) * 128,
                                             mg * 512:(mg + 1) * 512])
                            for mloc in range(4):
                                ct = mg * 4 + mloc
                                for nchk in range(NCHK):
                                    ps = pmm.tile([P, 512], F32, tag="pmm",
                                                  name=f"psq2_{ct}_{nchk}")
                                    for kt in range(CT):
                                        nc.tensor.matmul(
                                            ps[:],
                                            wt[:, kt,
                                               mloc * 128:(mloc + 1) * 128],
                                            hT[:, kt,
                                               nchk * 512:(nchk + 1) * 512],
                                            start=(kt == 0),
                                            stop=(kt == CT - 1))
                                    nc.any.tensor_copy(
                                        qcT_t[ct][:, nchk * 512:
                                                  (nchk + 1) * 512], ps[:])

                with tc.tile_pool(name="PT2", bufs=8) as pt_pool, \
                     tc.tile_pool(name="rcp2", bufs=4) as rcp_pool, \
                     tc.tile_pool(name="bcs2", bufs=4) as bcs_pool:
                    for ct in range(CT):
                        for qc in range(NCHK):
                            for hp in range(2):
                                h = 2 * ct + hp
                                pb = hp * 64
                                ps = ps_s.tile([P, 512], F32, tag="ps_s",
                                               name=f"Sc{h}_{qc}")
                                nc.tensor.matmul(
                                    ps[:], kcT[pb:pb + 64, ct, :],
                                    qcT_t[ct][pb:pb + 64,
                                              qc * 512:(qc + 1) * 512],
                                    start=True, stop=True)
                                pt = pt_pool.tile([P, 512], BF16, tag="pt2",
                                                  name=f"Pc{h}_{qc}")
                                nc.scalar.activation(pt[:], ps[:], AF.Exp,
                                                     scale=SCALE,
                                                     bias=maskb[:])
                                av = ps_av.tile([65, 512], F32, tag="ps_av",
                                                name=f"avc{h}_{qc}")
                                nc.tensor.matmul(av[:], vca[:, h, :], pt[:],
                                                 start=True, stop=True)
                                rc = rcp_pool.tile([1, 512], F32, tag="rc2",
                                                   name=f"rcc{h}_{qc}")
                                nc.vector.reciprocal(rc[:], av[64:65, :])
                                bc = bcs_pool.tile([64, 512], F32, tag="bc2",
                                                   name=f"bcc{h}_{qc}")
                                nc.gpsimd.partition_broadcast(bc[:], rc[:])
                                nc.vector.tensor_mul(
                                    caT_t[ct][pb:pb + 64,
                                              qc * 512:(qc + 1) * 512],
                                    av[0:64, :], bc[:])

                with tc.tile_pool(name="wpca", bufs=1) as wpca_pool:
                    wp = wpca_pool.tile([P, CT, C], BF16, name="wpca")
                    for kt in range(CT):
                        nc.sync.dma_start(
                            out=wp[:, kt, :],
                            in_=wpca_d[kt * 128:(kt + 1) * 128, :])
                    for tt in range(NT):
                        for nchk in range(NCHK):
                            ps = pmm.tile([P, 512], F32, tag="pmm",
                                          name=f"psr{tt}_{nchk}")
                            for kt in range(CT):
                                nc.tensor.matmul(
                                    ps[:],
                                    caT_t[kt][:, tt * 128:(tt + 1) * 128],
                                    wp[:, kt, nchk * 512:(nchk + 1) * 512],
                                    start=(kt == 0), stop=(kt == CT - 1))
                            sl = slice(nchk * 512, (nchk + 1) * 512)
                            nc.vector.tensor_add(x_sb[tt][:, sl],
                                                 x_sb[tt][:, sl], ps[:])

            # ================= MLP =================
            with tc.tile_pool(name="gT", bufs=1) as gT_pool:
                gT = gT_pool.tile([P, MT, N], BF16, name="gT")
                with tc.tile_pool(name="xn3", bufs=1) as xn_pool, \
                     tc.tile_pool(name="hT3", bufs=1) as hT_pool:
                    hT = hT_pool.tile([P, CT, N], BF16, name="hT3")
                    ln_mod_transpose(2, xn_pool, hT)
                    with tc.tile_pool(name="wfc1", bufs=2) as wfc1_pool, \
                         tc.tile_pool(name="gtmp", bufs=2) as gtmp_pool:
                        def g_evict(m, nchk, ps):
                            dst = gT[:, m, nchk * 512:(nchk + 1) * 512]
                            if not gelu_composite:
                                nc.scalar.activation(dst, ps[:],
                                                     AF.Gelu_apprx_tanh)
                                return
                            # sim-only composite gelu (CoreSim lacks the LUT)
                            sq = gtmp_pool.tile([P, 512], F32, tag="gsq",
                                                name=f"gsq{m}_{nchk}")
                            nc.scalar.activation(sq[:], ps[:], AF.Square)
                            x3 = gtmp_pool.tile([P, 512], F32, tag="gx3",
                                                name=f"gx3{m}_{nchk}")
                            nc.vector.tensor_mul(x3[:], sq[:], ps[:])
                            nc.vector.tensor_scalar(
                                out=x3[:], in0=x3[:], scalar1=0.044715,
                                scalar2=None, op0=ALU.mult)
                            nc.vector.tensor_add(x3[:], x3[:], ps[:])
                            th = gtmp_pool.tile([P, 512], F32, tag="gth",
                                                name=f"gth{m}_{nchk}")
                            nc.scalar.activation(th[:], x3[:], AF.Tanh,
                                                 scale=0.7978845608028654)
                            nc.vector.tensor_scalar(
                                out=th[:], in0=th[:], scalar1=0.5,
                                scalar2=0.5, op0=ALU.mult, op1=ALU.add)
                            nc.vector.tensor_mul(dst, th[:], ps[:])

                        for cg in range(8):
                            wt = wfc1_pool.tile([P, CT, 512], BF16,
                                                tag="wfc1", name=f"wfc1{cg}")
                            for kt in range(CT):
                                nc.sync.dma_start(
                                    out=wt[:, kt, :],
                                    in_=wfc1_d[cg, kt * 128:(kt + 1) * 128,
                                               :])
                            for mloc in range(4):
                                m = cg * 4 + mloc
                                for nchk in range(NCHK):
                                    ps = pmm.tile([P, 512], F32, tag="pmm",
                                                  name=f"psf1_{m}_{nchk}")
                                    for kt in range(CT):
                                        nc.tensor.matmul(
                                            ps[:],
                                            wt[:, kt,
                                               mloc * 128:(mloc + 1) * 128],
                                            hT[:, kt,
                                               nchk * 512:(nchk + 1) * 512],
                                            start=(kt == 0),
                                            stop=(kt == CT - 1))
                                    g_evict(m, nchk, ps)

                with tc.tile_pool(name="wfc2", bufs=2) as wfc2_pool:
                    for half in range(2):
                        wf = wfc2_pool.tile([P, MT, 512], BF16, tag="wfc2",
                                            name=f"wfc2_{half}")
                        for kt in range(MT):
                            nc.sync.dma_start(
                                out=wf[:, kt, :],
                                in_=wfc2_d[kt * 128:(kt + 1) * 128,
                                           half * 512:(half + 1) * 512])
                        for tt in range(NT):
                            ps = pmm.tile([P, 512], F32, tag="pmm",
                                          name=f"psf2_{half}_{tt}")
                            for kt in range(MT):
                                nc.tensor.matmul(
                                    ps[:], gT[:, kt, tt * 128:(tt + 1) * 128],
                                    wf[:, kt, :],
                                    start=(kt == 0), stop=(kt == MT - 1))
                            sl = slice(half * 512, (half + 1) * 512)
                            nc.vector.tensor_add(x_sb[tt][:, sl],
                                                 x_sb[tt][:, sl], ps[:])

            for tt in range(NT):
                nc.sync.dma_start(out=out_d[tt * 128:(tt + 1) * 128, :],
                                  in_=x_sb[tt][:])

        if repeat == 1:
            emit()
        else:
            with tc.For_i(0, repeat, 1):
                emit()

    nc.compile()
    return nc


def _prep_in_maps(inputs):
    bf = ml_dtypes.bfloat16
    x = np.asarray(inputs["x"], np.float32)
    c_dino = np.asarray(inputs["c_dino"], np.float32)
    c_text = np.asarray(inputs["c_text"], np.float32)
    mask = np.asarray(inputs["text_mask"])
    w = {k: np.ascontiguousarray(np.asarray(inputs[k], np.float32).astype(bf))
         for k in ["W_ada", "W_qkv", "W_proj_sa", "W_q", "W_kv", "W_proj_ca",
                   "W_fc1", "W_fc2"]}
    wqkv = w["W_qkv"]
    wqk = np.ascontiguousarray(
        np.stack([wqkv[:, 0:512], wqkv[:, 1024:1536],
                  wqkv[:, 512:1024], wqkv[:, 1536:2048]], axis=0))
    wv = np.ascontiguousarray(wqkv[:, 2048:3072])
    wfc1 = np.ascontiguousarray(
        np.stack([w["W_fc1"][:, i * 512:(i + 1) * 512] for i in range(8)],
                 axis=0))
    in_maps = []
    for i in range(N_CORES):
        in_maps.append({
            "x": np.ascontiguousarray(x[i]),
            "c_dino": np.ascontiguousarray(c_dino[i]),
            "c_textT": np.ascontiguousarray(c_text[i].T.astype(bf)),
            "mask_bias": np.ascontiguousarray(
                ((mask[i].astype(np.float32) - 1.0) * 30000.0).reshape(M, 1)),
            "Wada": w["W_ada"], "Wqk": wqk, "Wv": wv,
            "Wpsa": w["W_proj_sa"], "Wq": w["W_q"], "Wkv": w["W_kv"],
            "Wpca": w["W_proj_ca"], "Wfc1": wfc1, "Wfc2": w["W_fc2"],
        })
    return in_maps


_NC_CACHE = {}


def get_nc(repeat=1):
    if repeat not in _NC_CACHE:
        _NC_CACHE[repeat] = build_kernel(repeat=repeat)
    return _NC_CACHE[repeat]


def kernel(**inputs):
    for bn in ["b_ada", "b_qkv", "b_proj_sa", "b_q", "b_kv", "b_proj_ca",
               "b_fc1", "b_fc2"]:
        if bn in inputs:
            assert float(np.abs(np.asarray(inputs[bn])).max()) == 0.0, \
                f"nonzero bias {bn} not supported"
    from concourse.bass_utils import run_bass_kernel_spmd
    nc = get_nc(repeat=1)
    in_maps = _prep_in_maps(inputs)
    res = run_bass_kernel_spmd(nc, in_maps, core_ids=list(range(N_CORES)))
    out = np.stack([res.results[i]["out"] for i in range(N_CORES)], axis=0)
    return out.astype(np.float32)
